# revision 23
# baseline (speedup 1.0000x reference)
"""Bidirectional Mamba layer for Trainium2 (8 NeuronCores).

Sharding: core = (batch b in {0,1}) x (direction in {fwd,bwd}) x (d_inner half).
All 8 cores run one SPMD program with per-core input arrays; there are no
cross-core collectives. The host flips the sequence for the backward direction,
permutes u-channels so each core's own d_inner half is always channel-tiles
0..5, pre-tiles every weight matrix so each SBUF destination loads with one
large contiguous DMA (the HWDGE unit costs ~625ns per DMA instruction), and
sums the row-parallel + fwd/bwd partial outputs during the gather.

Per-core program:
  A) in_proj (fp32r matmuls), causal depthwise conv as 4 diagonal-matmul taps
     on the tensor engine (diagonals built on the idle vector engine), SiLU;
     xproj accumulated incrementally as each u-tile is produced;
     softplus(dt_proj + bias) via exp+ln; w = delta*u.
  B) selective scan: for each (d-tile, state n): dA = exp(delta * A[:,n]) on
     the scalar engine, dBu = w * bcast(B_n) on vector, hardware
     tensor_tensor_scan over t, g = h * bcast(C_n), and y += I.T @ g
     accumulated in PSUM by the tensor engine (the sum over n).
  C) y = (y + u*D) * silu(z);  D) out_proj partial, summed on host.
"""
import sys

sys.path.insert(0, "/opt/trn_rl_repo")

from contextlib import ExitStack

import numpy as np

import concourse.bass as bass
import concourse.mybir as mybir
import concourse.tile as tile
from concourse import bacc
from concourse.bass_utils import run_bass_kernel_spmd

D_MODEL = 768
D_STATE = 16
D_INNER = 1536
DT_RANK = 48
D_CONV = 4
BATCH = 2
SEQ = 1024
DH = D_INNER // 2          # 768 scan channels per core
P = 128
KM = D_MODEL // P          # 6 k-tiles over d_model
MU = D_INNER // P          # 12 m-tiles for full u
MH = DH // P               # 6 m-tiles for the half (z, delta, scan, out_proj k)
TH = SEQ // 512            # 2 t-halves for matmul free dim

F32 = mybir.dt.float32
F32R = mybir.dt.float32r
BF16 = mybir.dt.bfloat16
AF = mybir.ActivationFunctionType
OP = mybir.AluOpType

_CACHE = {}


def _build():
    nc = bacc.Bacc("TRN2", target_bir_lowering=False, debug=False)

    xT = nc.dram_tensor("xT", [P, KM, SEQ], F32R, kind="ExternalInput")
    wuX = nc.dram_tensor("wuX", [MU, P, KM * P], F32R, kind="ExternalInput")
    wzX = nc.dram_tensor("wzX", [MH, P, KM * P], F32R, kind="ExternalInput")
    convw = nc.dram_tensor("convw", [P, MU, D_CONV], F32, kind="ExternalInput")
    cbias = nc.dram_tensor("cbias", [P, MU], F32, kind="ExternalInput")
    xpX = nc.dram_tensor("xpX", [P, MU, 80], F32R, kind="ExternalInput")
    dtwT = nc.dram_tensor("dtwT", [DT_RANK + 1, DH], F32R, kind="ExternalInput")
    ones1 = nc.dram_tensor("ones1", [1, SEQ], F32R, kind="ExternalInput")
    Amat = nc.dram_tensor("Amat", [P, MH, D_STATE], F32, kind="ExternalInput")
    Dsk = nc.dram_tensor("Dsk", [P, MH], F32, kind="ExternalInput")
    owX = nc.dram_tensor("owX", [P, MH, KM, P], F32R, kind="ExternalInput")
    eye = nc.dram_tensor("eye", [P, P], F32R, kind="ExternalInput")
    zpad = nc.dram_tensor("zpad", [P, D_CONV - 1], F32R, kind="ExternalInput")
    outp = nc.dram_tensor("outp", [D_MODEL, SEQ], F32, kind="ExternalOutput")

    with tile.TileContext(nc) as tc, ExitStack() as top:
        persist = top.enter_context(tc.tile_pool(name="persist", bufs=1))
        ops_pool = top.enter_context(tc.tile_pool(name="ps_o", bufs=2, space="PSUM"))
        dram = top.enter_context(tc.tile_pool(name="dram", bufs=1, space="DRAM"))
        us = [persist.tile([P, SEQ], F32R, tag=f"us{m}", name=f"us{m}")
              for m in range(MH)]
        sz = [persist.tile([P, SEQ], F32, tag=f"sz{m}", name=f"sz{m}")
              for m in range(MH)]
        delta_all = persist.tile([P, MH, SEQ], BF16, tag="dl")
        wdu = [persist.tile([P, SEQ], BF16, tag=f"w{m}", name=f"w{m}")
               for m in range(MH)]
        A_sb = persist.tile([P, MH, D_STATE], F32, tag="A")
        cb_sb = persist.tile([P, MU], F32, tag="cb")
        dsk_sb = persist.tile([P, MH], F32, tag="dsk")
        cw_sb = persist.tile([P, MU, D_CONV], F32, tag="cw")
        eye_sb = persist.tile([P, P], F32R, tag="eye")
        ow_sb = persist.tile([P, MH, KM, P], F32R, tag="ow")
        eye_b = persist.tile([P, P], BF16, tag="eyeb")
        bcd = dram.tile([2 * D_STATE, SEQ], BF16, tag="bc")
        nc.sync.dma_start(out=A_sb, in_=Amat[:, :, :])
        nc.sync.dma_start(out=dsk_sb, in_=Dsk[:, :])
        nc.sync.dma_start(out=cb_sb, in_=cbias[:, :])
        nc.sync.dma_start(out=cw_sb, in_=convw[:, :, :])
        nc.sync.dma_start(out=eye_sb, in_=eye[:, :])

        # ---------------- Phase A: projections ----------------
        with ExitStack() as pa:
            xs_pool = top.enter_context(tc.tile_pool(name="xs", bufs=1))
            wpool = top.enter_context(tc.tile_pool(name="wstream", bufs=4))
            djpool = pa.enter_context(tc.tile_pool(name="djp", bufs=8))
            ubuf_pool = pa.enter_context(tc.tile_pool(name="ubuf", bufs=1))
            uoth_pool = pa.enter_context(tc.tile_pool(name="uoth", bufs=2))
            ps_a = pa.enter_context(tc.tile_pool(name="ps_a", bufs=2, space="PSUM"))
            ps_xp = pa.enter_context(tc.tile_pool(name="ps_xp", bufs=1, space="PSUM"))
            misc = pa.enter_context(tc.tile_pool(name="misc_a", bufs=1))

            xs_all = xs_pool.tile([P, KM, SEQ], F32R, tag="xs")
            nc.sync.dma_start(out=xs_all, in_=xT[:, :, :])
            xs = [xs_all[:, k, :] for k in range(KM)]

            xp_all = misc.tile([P, MU, 80], F32R, tag="xp")
            nc.sync.dma_start(out=xp_all, in_=xpX[:, :, :])

            # two conv staging buffers; zero pad written once each
            ubufs = [ubuf_pool.tile([P, D_CONV - 1 + SEQ], F32R, tag=f"ubuf{i}",
                                    name=f"ubuf{i}") for i in range(2)]
            for i in range(2):
                nc.sync.dma_start(out=ubufs[i][:, 0:D_CONV - 1], in_=zpad[:, :])

            # xproj accumulators, fed incrementally as each u-tile is made
            psx = [ps_xp.tile([80, 512], F32, tag=f"psx{th}", name=f"psx{th}")
                   for th in range(TH)]

            # u path: in_proj -> causal conv -> silu -> xproj contribution
            for m in range(MU):
                wu_m = wpool.tile([P, KM * P], F32R, tag="w")
                nc.sync.dma_start(out=wu_m, in_=wuX[m, :, :])
                ub = ubufs[m % 2]
                for th in range(TH):
                    ps = ps_a.tile([P, 512], F32, tag="ps")
                    for k in range(KM):
                        nc.tensor.matmul(ps, wu_m[:, k * P:(k + 1) * P],
                                         xs[k][:, th * 512:(th + 1) * 512],
                                         start=(k == 0), stop=(k == KM - 1))
                    nc.scalar.copy(
                        out=ub[:, D_CONV - 1 + th * 512:D_CONV - 1 + (th + 1) * 512],
                        in_=ps)
                # depthwise causal conv as 4 diagonal-matmul taps;
                # diagonals built on the (idle) vector engine
                ut = us[m] if m < MH else uoth_pool.tile([P, SEQ], F32R,
                                                         tag="uo", name="uo")
                djs = []
                for j in range(D_CONV):
                    dj = djpool.tile([P, P], F32R, tag="dj")
                    nc.vector.tensor_scalar_mul(dj, eye_sb, cw_sb[:, m, j:j + 1])
                    djs.append(dj)
                for th in range(TH):
                    psc = ps_a.tile([P, 512], F32, tag="ps")
                    for j in range(D_CONV):
                        nc.tensor.matmul(psc, djs[j],
                                         ub[:, j + th * 512:j + th * 512 + 512],
                                         start=(j == 0), stop=(j == D_CONV - 1))
                    nc.scalar.activation(out=ut[:, th * 512:(th + 1) * 512], in_=psc,
                                         func=AF.Silu, bias=cb_sb[:, m:m + 1])
                # xproj: accumulate this k=m contribution into psx
                for th in range(TH):
                    nc.tensor.matmul(psx[th], xp_all[:, m, :],
                                     ut[:, th * 512:(th + 1) * 512],
                                     start=(m == 0), stop=(m == MU - 1))

            # z path: in_proj half + silu (PE fills the delta/ACT window)
            for mz in range(MH):
                wz_m = wpool.tile([P, KM * P], F32R, tag="w")
                nc.sync.dma_start(out=wz_m, in_=wzX[mz, :, :])
                for th in range(TH):
                    ps = ops_pool.tile([P, 512], F32, tag="ps")
                    for k in range(KM):
                        nc.tensor.matmul(ps, wz_m[:, k * P:(k + 1) * P],
                                         xs[k][:, th * 512:(th + 1) * 512],
                                         start=(k == 0), stop=(k == KM - 1))
                    nc.scalar.activation(out=sz[mz][:, th * 512:(th + 1) * 512],
                                         in_=ps, func=AF.Silu)

            # x_dbl out of PSUM: fp32 copy (B/C rows) + fp32r copy (dt rows)
            xd_bc = misc.tile([80, SEQ], BF16, tag="xdbc")
            xd_r = misc.tile([DT_RANK + 1, SEQ], F32R, tag="xdr")
            for th in range(TH):
                # non-zero-base partition slices are limited to 32 partitions
                nc.scalar.copy(out=xd_bc[32:64, th * 512:(th + 1) * 512],
                               in_=psx[th][32:64, :])
                nc.scalar.copy(out=xd_bc[64:80, th * 512:(th + 1) * 512],
                               in_=psx[th][64:80, :])
                nc.scalar.copy(out=xd_r[0:DT_RANK, th * 512:(th + 1) * 512],
                               in_=psx[th][0:DT_RANK, :])

            # delta = softplus(dt @ dt_w.T + dt_b) = ln(exp(.) + 1), batched:
            # dt_b rides as an extra contraction row against a ones-row, so
            # exp/ln run as two whole-width ACT ops (no table thrash)
            nc.sync.dma_start(out=xd_r[DT_RANK:DT_RANK + 1, :], in_=ones1[:, :])
            dtw_sb = misc.tile([DT_RANK + 1, DH], F32R, tag="dtw")
            nc.sync.dma_start(out=dtw_sb, in_=dtwT[:, :])
            ps_dt = pa.enter_context(tc.tile_pool(name="ps_dt", bufs=1,
                                                  space="PSUM"))
            for th in range(TH):
                e1 = misc.tile([P, MH, 512], BF16, tag="sp_e", bufs=2)
                for mb in range(MH // 2):
                    psd2 = ps_dt.tile([P, 2, 512], F32, tag="psd")
                    for mi in range(2):
                        m = 2 * mb + mi
                        nc.tensor.matmul(psd2[:, mi, :],
                                         dtw_sb[:, m * P:(m + 1) * P],
                                         xd_r[:, th * 512:(th + 1) * 512],
                                         start=True, stop=True)
                    nc.scalar.activation(out=e1[:, 2 * mb:2 * mb + 2, :],
                                         in_=psd2, func=AF.Exp)
                nc.scalar.activation(
                    out=delta_all[:, :, th * 512:(th + 1) * 512],
                    in_=e1, func=AF.Ln, bias=1.0)

            # w = delta * u  (scan-half channels only)
            for m in range(MH):
                nc.vector.tensor_tensor(out=wdu[m], in0=delta_all[:, m, :],
                                        in1=us[m], op=OP.mult)

            # stage B and C rows to DRAM for partition-broadcast reads
            nc.sync.dma_start(out=bcd[:, :], in_=xd_bc[DT_RANK:80, :])

        nc.sync.dma_start(out=ow_sb, in_=owX[:, :, :, :])
        nc.scalar.copy(out=eye_b, in_=eye_sb)

        late = top.enter_context(tc.tile_pool(name="late", bufs=1))
        yf = [late.tile([P, SEQ], F32R, tag=f"yf{m}", name=f"yf{m}")
              for m in range(MH)]

        # ---------------- Phase B: selective scan ----------------
        with ExitStack() as pb:
            bc_pool = pb.enter_context(tc.tile_pool(name="bc", bufs=2))
            sc_pool = pb.enter_context(tc.tile_pool(name="scan", bufs=2))
            ps_y = pb.enter_context(tc.tile_pool(name="ps_y", bufs=1, space="PSUM"))
            NDSET = 2
            DPS = MH // NDSET  # 3 d-tiles per set
            for ds in range(NDSET):
                yps = [ps_y.tile([P, SEQ], F32, tag=f"y{i}", name=f"y{i}")
                       for i in range(DPS)]
                NG = 2
                for np_ in range(D_STATE // NG):
                    n0 = NG * np_
                    # rows {n0..n0+3} and {16+n0..}: [bc-pair, n-group, t]
                    bcg = bc_pool.tile([P, 2, NG, SEQ], BF16, tag="bc2")
                    srcg = bass.AP(
                        tensor=bcd.tensor, offset=bcd.offset + n0 * SEQ,
                        ap=[[0, P], [D_STATE * SEQ, 2], [SEQ, NG], [1, SEQ]])
                    nc.sync.dma_start(out=bcg, in_=srcg)
                    for i in range(DPS):
                        m = ds * DPS + i
                        dbu4 = sc_pool.tile([P, NG, SEQ], BF16, tag="dbu")
                        nc.vector.tensor_tensor(
                            out=dbu4,
                            in0=wdu[m].unsqueeze(1).broadcast_to([P, NG, SEQ]),
                            in1=bcg[:, 0, :, :], op=OP.mult)
                        h4 = sc_pool.tile([P, NG, SEQ], BF16, tag="h")
                        for j in range(NG):
                            da = sc_pool.tile([P, SEQ], BF16, tag="da")
                            nc.scalar.activation(out=da, in_=delta_all[:, m, :],
                                                 func=AF.Exp,
                                                 scale=A_sb[:, m, n0 + j:n0 + j + 1])
                            nc.vector.tensor_tensor_scan(
                                out=h4[:, j, :], data0=da, data1=dbu4[:, j, :],
                                initial=0.0, op0=OP.mult, op1=OP.add)
                        g4 = sc_pool.tile([P, NG, SEQ], BF16, tag="g")
                        nc.vector.tensor_tensor(out=g4, in0=h4,
                                                in1=bcg[:, 1, :, :], op=OP.mult)
                        for j in range(NG):
                            for th in range(TH):
                                nc.tensor.matmul(
                                    yps[i][:, th * 512:(th + 1) * 512], eye_b,
                                    g4[:, j, th * 512:(th + 1) * 512],
                                    start=(n0 + j == 0), stop=False)
                # Phase C for this d-set: y += u*D on PE, then gate with silu(z)
                for i in range(DPS):
                    m = ds * DPS + i
                    dD = sc_pool.tile([P, P], F32R, tag="dD", bufs=3)
                    nc.vector.tensor_scalar_mul(dD, eye_sb, dsk_sb[:, m:m + 1])
                    for th in range(TH):
                        nc.tensor.matmul(yps[i][:, th * 512:(th + 1) * 512], dD,
                                         us[m][:, th * 512:(th + 1) * 512],
                                         start=False, stop=True)
                    nc.vector.tensor_tensor(out=yf[m], in0=yps[i], in1=sz[m],
                                            op=OP.mult)

        # ---------------- Phase D: out_proj ----------------
        with ExitStack() as pd:
            ost = pd.enter_context(tc.tile_pool(name="ost", bufs=2))
            for m in range(KM):
                ot = ost.tile([P, SEQ], F32, tag="ot")
                for th in range(TH):
                    ps = ops_pool.tile([P, 512], F32, tag="ps")
                    for k in range(MH):
                        nc.tensor.matmul(ps, ow_sb[:, k, m, :],
                                         yf[k][:, th * 512:(th + 1) * 512],
                                         start=(k == 0), stop=(k == MH - 1))
                    nc.scalar.copy(out=ot[:, th * 512:(th + 1) * 512], in_=ps)
                nc.sync.dma_start(out=outp[m * P:(m + 1) * P, :], in_=ot)

    nc.finalize()
    return nc


def _prep_core(x, prm, b, direction, half):
    """Build the per-core input map. prm maps param name -> array."""
    xb = np.ascontiguousarray(x[b])                # (L, D_MODEL)
    if direction == 1:
        xb = np.ascontiguousarray(xb[::-1])
    in_w = prm["in_w"]
    conv_w = prm["conv_w"]
    conv_b = prm["conv_b"]
    xproj_w = prm["xproj_w"]
    dt_w = prm["dt_w"]
    dt_b = prm["dt_b"]
    Alog = prm["Alog"]
    Dp = prm["D"]
    out_w = prm["out_w"]

    own = np.arange(half * DH, (half + 1) * DH)
    oth = np.arange((1 - half) * DH, (2 - half) * DH)
    perm = np.concatenate([own, oth])              # u-channel permutation

    wu = in_w[0:D_INNER][perm]                     # (1536, 768), own half first
    wz = in_w[D_INNER:2 * D_INNER][own]            # (768, 768)
    cw = conv_w[perm]                              # (1536, 4)
    A = -np.exp(Alog[own])                         # (768, 16)

    def lhs_tiles(mat_t, kk, mm):
        # (K*P, M*P) -> (mm, P, kk*P): per m-tile, partition-contiguous rows
        return np.ascontiguousarray(
            mat_t.reshape(kk, P, mm, P).transpose(2, 1, 0, 3).reshape(mm, P, kk * P))

    return {
        "xT": np.ascontiguousarray(xb.T.reshape(KM, P, SEQ).transpose(1, 0, 2)),
        "wuX": lhs_tiles(wu.T, KM, MU),
        "wzX": lhs_tiles(wz.T, KM, MH),
        "convw": np.ascontiguousarray(cw.reshape(MU, P, D_CONV).transpose(1, 0, 2)),
        "cbias": np.ascontiguousarray(conv_b[perm].reshape(MU, P).T),
        "xpX": np.ascontiguousarray(
            xproj_w[:, perm].T.reshape(MU, P, 80).transpose(1, 0, 2)),
        "dtwT": np.ascontiguousarray(
            np.vstack([dt_w[own].T, dt_b[own][None, :]])),
        "ones1": np.ones((1, SEQ), dtype=np.float32),
        "Amat": np.ascontiguousarray(A.reshape(MH, P, D_STATE).transpose(1, 0, 2)),
        "Dsk": np.ascontiguousarray(Dp[own].reshape(MH, P).T),
        "owX": np.ascontiguousarray(
            out_w[:, own].T.reshape(MH, P, KM, P).transpose(1, 0, 2, 3)),
        "eye": np.eye(P, dtype=np.float32),
        "zpad": np.zeros((P, D_CONV - 1), dtype=np.float32),
    }


def _in_maps(inputs):
    x = inputs["x"]
    maps = []
    for b in range(BATCH):
        for direction in range(2):
            pfx = "f" if direction == 0 else "b"
            prm = {k: inputs[f"{pfx}_{k}"] for k in
                   ("in_w", "conv_w", "conv_b", "xproj_w", "dt_w", "dt_b",
                    "Alog", "D", "out_w")}
            for half in range(2):
                maps.append(_prep_core(x, prm, b, direction, half))
    return maps


def kernel(**inputs):
    inputs = {k: np.asarray(v, dtype=np.float32) for k, v in inputs.items()}
    nc = _CACHE.get("nc")
    if nc is None:
        nc = _build()
        _CACHE["nc"] = nc
    maps = _in_maps(inputs)
    res = run_bass_kernel_spmd(nc, maps, list(range(8)),
                               **_CACHE.get("run_kwargs", {}))
    _CACHE["last_results"] = res
    out = np.zeros((BATCH, SEQ, D_MODEL), dtype=np.float32)
    ci = 0
    for b in range(BATCH):
        for direction in range(2):
            for half in range(2):
                part = res.results[ci]["outp"].T          # (SEQ, D_MODEL)
                if direction == 1:
                    part = part[::-1]
                out[b] += part
                ci += 1
    return out


# revision 24
# speedup vs baseline: 1.0062x; 1.0062x over previous
"""Bidirectional Mamba layer for Trainium2 (8 NeuronCores).

Sharding: core = (batch b in {0,1}) x (direction in {fwd,bwd}) x (d_inner half).
All 8 cores run one SPMD program with per-core input arrays; there are no
cross-core collectives. The host flips the sequence for the backward direction,
permutes u-channels so each core's own d_inner half is always channel-tiles
0..5, pre-tiles every weight matrix so each SBUF destination loads with one
large contiguous DMA (the HWDGE unit costs ~625ns per DMA instruction), and
sums the row-parallel + fwd/bwd partial outputs during the gather.

Per-core program:
  A) in_proj (fp32r matmuls), causal depthwise conv as 4 diagonal-matmul taps
     on the tensor engine (diagonals built on the idle vector engine), SiLU;
     xproj accumulated incrementally as each u-tile is produced;
     softplus(dt_proj + bias) via exp+ln; w = delta*u.
  B) selective scan: for each (d-tile, state n): dA = exp(delta * A[:,n]) on
     the scalar engine, dBu = w * bcast(B_n) on vector, hardware
     tensor_tensor_scan over t, g = h * bcast(C_n), and y += I.T @ g
     accumulated in PSUM by the tensor engine (the sum over n).
  C) y = (y + u*D) * silu(z);  D) out_proj partial, summed on host.
"""
import sys

sys.path.insert(0, "/opt/trn_rl_repo")

from contextlib import ExitStack

import ml_dtypes
import numpy as np

import concourse.bass as bass
import concourse.mybir as mybir
import concourse.tile as tile
from concourse import bacc
from concourse.bass_utils import run_bass_kernel_spmd

D_MODEL = 768
D_STATE = 16
D_INNER = 1536
DT_RANK = 48
D_CONV = 4
BATCH = 2
SEQ = 1024
DH = D_INNER // 2          # 768 scan channels per core
P = 128
KM = D_MODEL // P          # 6 k-tiles over d_model
MU = D_INNER // P          # 12 m-tiles for full u
MH = DH // P               # 6 m-tiles for the half (z, delta, scan, out_proj k)
TH = SEQ // 512            # 2 t-halves for matmul free dim

F32 = mybir.dt.float32
F32R = mybir.dt.float32r
BF16 = mybir.dt.bfloat16
AF = mybir.ActivationFunctionType
OP = mybir.AluOpType

_CACHE = {}


def _build():
    nc = bacc.Bacc("TRN2", target_bir_lowering=False, debug=False)

    xT = nc.dram_tensor("xT", [P, KM, SEQ], F32R, kind="ExternalInput")
    wuX = nc.dram_tensor("wuX", [MU, P, KM * P], F32R, kind="ExternalInput")
    wzX = nc.dram_tensor("wzX", [MH, P, KM * P], F32R, kind="ExternalInput")
    convw = nc.dram_tensor("convw", [P, MU, D_CONV], F32, kind="ExternalInput")
    cbias = nc.dram_tensor("cbias", [P, MU], F32, kind="ExternalInput")
    xpX = nc.dram_tensor("xpX", [P, MU, 80], F32R, kind="ExternalInput")
    dtwT = nc.dram_tensor("dtwT", [DT_RANK + 1, DH], F32R, kind="ExternalInput")
    ones1 = nc.dram_tensor("ones1", [1, SEQ], F32R, kind="ExternalInput")
    Amat = nc.dram_tensor("Amat", [P, MH, D_STATE], F32, kind="ExternalInput")
    Dsk = nc.dram_tensor("Dsk", [P, MH], F32, kind="ExternalInput")
    owX = nc.dram_tensor("owX", [P, MH, KM, P], F32R, kind="ExternalInput")
    eye = nc.dram_tensor("eye", [P, P], F32R, kind="ExternalInput")
    zpad = nc.dram_tensor("zpad", [P, D_CONV - 1], F32R, kind="ExternalInput")
    zb = nc.dram_tensor("zb", [P, 2], BF16, kind="ExternalInput")
    outp = nc.dram_tensor("outp", [D_MODEL, SEQ], F32, kind="ExternalOutput")

    with tile.TileContext(nc) as tc, ExitStack() as top:
        persist = top.enter_context(tc.tile_pool(name="persist", bufs=1))
        ops_pool = top.enter_context(tc.tile_pool(name="ps_o", bufs=2, space="PSUM"))
        dram = top.enter_context(tc.tile_pool(name="dram", bufs=1, space="DRAM"))
        us = [persist.tile([P, SEQ], F32R, tag=f"us{m}", name=f"us{m}")
              for m in range(MH)]
        sz = [persist.tile([P, SEQ], F32, tag=f"sz{m}", name=f"sz{m}")
              for m in range(MH)]
        delta_all = persist.tile([P, MH, SEQ], BF16, tag="dl")
        wdu = [persist.tile([P, SEQ], BF16, tag=f"w{m}", name=f"w{m}")
               for m in range(MH)]
        A_sb = persist.tile([P, MH, D_STATE], F32, tag="A")
        cb_sb = persist.tile([P, MU], F32, tag="cb")
        dsk_sb = persist.tile([P, MH], F32, tag="dsk")
        cw_sb = persist.tile([P, MU, D_CONV], F32, tag="cw")
        eye_sb = persist.tile([P, P], F32R, tag="eye")
        ow_sb = persist.tile([P, MH, KM, P], F32R, tag="ow")
        eye_b = persist.tile([P, P], BF16, tag="eyeb")
        bcd = dram.tile([2 * D_STATE, SEQ], BF16, tag="bc")
        nc.sync.dma_start(out=A_sb, in_=Amat[:, :, :])
        nc.sync.dma_start(out=dsk_sb, in_=Dsk[:, :])
        nc.sync.dma_start(out=cb_sb, in_=cbias[:, :])
        nc.sync.dma_start(out=cw_sb, in_=convw[:, :, :])
        nc.sync.dma_start(out=eye_sb, in_=eye[:, :])

        # ---------------- Phase A: projections ----------------
        with ExitStack() as pa:
            xs_pool = top.enter_context(tc.tile_pool(name="xs", bufs=1))
            wpool = top.enter_context(tc.tile_pool(name="wstream", bufs=4))
            djpool = pa.enter_context(tc.tile_pool(name="djp", bufs=8))
            ubuf_pool = pa.enter_context(tc.tile_pool(name="ubuf", bufs=1))
            uoth_pool = pa.enter_context(tc.tile_pool(name="uoth", bufs=2))
            ps_a = pa.enter_context(tc.tile_pool(name="ps_a", bufs=2, space="PSUM"))
            ps_xp = pa.enter_context(tc.tile_pool(name="ps_xp", bufs=1, space="PSUM"))
            misc = pa.enter_context(tc.tile_pool(name="misc_a", bufs=1))

            xs_all = xs_pool.tile([P, KM, SEQ], F32R, tag="xs")
            nc.sync.dma_start(out=xs_all, in_=xT[:, :, :])
            xs = [xs_all[:, k, :] for k in range(KM)]

            xp_all = misc.tile([P, MU, 80], F32R, tag="xp")
            nc.sync.dma_start(out=xp_all, in_=xpX[:, :, :])

            # two conv staging buffers; zero pad written once each
            ubufs = [ubuf_pool.tile([P, D_CONV - 1 + SEQ], F32R, tag=f"ubuf{i}",
                                    name=f"ubuf{i}") for i in range(2)]
            for i in range(2):
                nc.sync.dma_start(out=ubufs[i][:, 0:D_CONV - 1], in_=zpad[:, :])

            # xproj accumulators, fed incrementally as each u-tile is made
            psx = [ps_xp.tile([80, 512], F32, tag=f"psx{th}", name=f"psx{th}")
                   for th in range(TH)]

            # u path: in_proj -> causal conv -> silu -> xproj contribution
            for m in range(MU):
                wu_m = wpool.tile([P, KM * P], F32R, tag="w")
                nc.sync.dma_start(out=wu_m, in_=wuX[m, :, :])
                ub = ubufs[m % 2]
                for th in range(TH):
                    ps = ps_a.tile([P, 512], F32, tag="ps")
                    for k in range(KM):
                        nc.tensor.matmul(ps, wu_m[:, k * P:(k + 1) * P],
                                         xs[k][:, th * 512:(th + 1) * 512],
                                         start=(k == 0), stop=(k == KM - 1))
                    nc.scalar.copy(
                        out=ub[:, D_CONV - 1 + th * 512:D_CONV - 1 + (th + 1) * 512],
                        in_=ps)
                # depthwise causal conv as 4 diagonal-matmul taps;
                # diagonals built on the (idle) vector engine
                ut = us[m] if m < MH else uoth_pool.tile([P, SEQ], F32R,
                                                         tag="uo", name="uo")
                djs = []
                for j in range(D_CONV):
                    dj = djpool.tile([P, P], F32R, tag="dj")
                    nc.vector.tensor_scalar_mul(dj, eye_sb, cw_sb[:, m, j:j + 1])
                    djs.append(dj)
                for th in range(TH):
                    psc = ps_a.tile([P, 512], F32, tag="ps")
                    for j in range(D_CONV):
                        nc.tensor.matmul(psc, djs[j],
                                         ub[:, j + th * 512:j + th * 512 + 512],
                                         start=(j == 0), stop=(j == D_CONV - 1))
                    nc.scalar.activation(out=ut[:, th * 512:(th + 1) * 512], in_=psc,
                                         func=AF.Silu, bias=cb_sb[:, m:m + 1])
                # xproj: accumulate this k=m contribution into psx
                for th in range(TH):
                    nc.tensor.matmul(psx[th], xp_all[:, m, :],
                                     ut[:, th * 512:(th + 1) * 512],
                                     start=(m == 0), stop=(m == MU - 1))

            # z path: in_proj half + silu (PE fills the delta/ACT window)
            for mz in range(MH):
                wz_m = wpool.tile([P, KM * P], F32R, tag="w")
                nc.sync.dma_start(out=wz_m, in_=wzX[mz, :, :])
                for th in range(TH):
                    ps = ops_pool.tile([P, 512], F32, tag="ps")
                    for k in range(KM):
                        nc.tensor.matmul(ps, wz_m[:, k * P:(k + 1) * P],
                                         xs[k][:, th * 512:(th + 1) * 512],
                                         start=(k == 0), stop=(k == KM - 1))
                    nc.scalar.activation(out=sz[mz][:, th * 512:(th + 1) * 512],
                                         in_=ps, func=AF.Silu)

            # x_dbl out of PSUM: fp32 copy (B/C rows) + fp32r copy (dt rows)
            xd_bc = misc.tile([80, SEQ], BF16, tag="xdbc")
            xd_r = misc.tile([DT_RANK + 1, SEQ], F32R, tag="xdr")
            for th in range(TH):
                # non-zero-base partition slices are limited to 32 partitions
                nc.scalar.copy(out=xd_bc[32:64, th * 512:(th + 1) * 512],
                               in_=psx[th][32:64, :])
                nc.scalar.copy(out=xd_bc[64:80, th * 512:(th + 1) * 512],
                               in_=psx[th][64:80, :])
                nc.scalar.copy(out=xd_r[0:DT_RANK, th * 512:(th + 1) * 512],
                               in_=psx[th][0:DT_RANK, :])

            # delta = softplus(dt @ dt_w.T + dt_b) = ln(exp(.) + 1), batched:
            # dt_b rides as an extra contraction row against a ones-row, so
            # exp/ln run as two whole-width ACT ops (no table thrash)
            nc.sync.dma_start(out=xd_r[DT_RANK:DT_RANK + 1, :], in_=ones1[:, :])
            dtw_sb = misc.tile([DT_RANK + 1, DH], F32R, tag="dtw")
            nc.sync.dma_start(out=dtw_sb, in_=dtwT[:, :])
            ps_dt = pa.enter_context(tc.tile_pool(name="ps_dt", bufs=1,
                                                  space="PSUM"))
            for th in range(TH):
                e1 = misc.tile([P, MH, 512], BF16, tag="sp_e", bufs=2)
                for mb in range(MH // 2):
                    psd2 = ps_dt.tile([P, 2, 512], F32, tag="psd")
                    for mi in range(2):
                        m = 2 * mb + mi
                        nc.tensor.matmul(psd2[:, mi, :],
                                         dtw_sb[:, m * P:(m + 1) * P],
                                         xd_r[:, th * 512:(th + 1) * 512],
                                         start=True, stop=True)
                    nc.scalar.activation(out=e1[:, 2 * mb:2 * mb + 2, :],
                                         in_=psd2, func=AF.Exp)
                nc.scalar.activation(
                    out=delta_all[:, :, th * 512:(th + 1) * 512],
                    in_=e1, func=AF.Ln, bias=1.0)

            # w = delta * u  (scan-half channels only)
            for m in range(MH):
                nc.vector.tensor_tensor(out=wdu[m], in0=delta_all[:, m, :],
                                        in1=us[m], op=OP.mult)

            # stage B and C rows to DRAM for partition-broadcast reads
            nc.sync.dma_start(out=bcd[:, :], in_=xd_bc[DT_RANK:80, :])

        nc.sync.dma_start(out=ow_sb, in_=owX[:, :, :, :])
        nc.scalar.copy(out=eye_b, in_=eye_sb)

        late = top.enter_context(tc.tile_pool(name="late", bufs=1))
        yf = [late.tile([P, SEQ], F32R, tag=f"yf{m}", name=f"yf{m}")
              for m in range(MH)]

        # ---------------- Phase B: selective scan ----------------
        _CACHE0 = {}
        with ExitStack() as pb:
            bc_pool = pb.enter_context(tc.tile_pool(name="bc", bufs=2))
            sc_pool = pb.enter_context(tc.tile_pool(name="scan", bufs=2))
            ps_y = pb.enter_context(tc.tile_pool(name="ps_y", bufs=1, space="PSUM"))
            NDSET = 2
            DPS = MH // NDSET  # 3 d-tiles per set
            for ds in range(NDSET):
                yps = [ps_y.tile([P, SEQ], F32, tag=f"y{i}", name=f"y{i}")
                       for i in range(DPS)]
                NG = 2
                for np_ in range(D_STATE // NG):
                    n0 = NG * np_
                    # rows {n0..n0+3} and {16+n0..}: [bc-pair, n-group, t]
                    bcg = bc_pool.tile([P, 2, NG, SEQ], BF16, tag="bc2")
                    srcg = bass.AP(
                        tensor=bcd.tensor, offset=bcd.offset + n0 * SEQ,
                        ap=[[0, P], [D_STATE * SEQ, 2], [SEQ, NG], [1, SEQ]])
                    nc.sync.dma_start(out=bcg, in_=srcg)
                    for i in range(DPS):
                        m = ds * DPS + i
                        # rows padded to SEQ+2 with zero boundary columns so a
                        # single chained scan covers both n's (state resets to
                        # zero through the dA=0, dBu=0 boundary elements);
                        # even row stride keeps bf16 ops 4B-aligned
                        SP2 = SEQ + 2
                        dbu4 = sc_pool.tile([P, NG, SP2], BF16, tag="dbu")
                        da4 = sc_pool.tile([P, NG, SP2], BF16, tag="da")
                        ctr = _CACHE0.setdefault("bz", 0)
                        if ctr < 2:
                            _CACHE0["bz"] = ctr + 1
                            for tzi in (dbu4, da4):
                                nc.sync.dma_start(
                                    out=tzi[:, :, SEQ:SP2],
                                    in_=zb[:, :].unsqueeze(1)
                                        .broadcast_to([P, NG, 2]))
                        nc.vector.tensor_tensor(
                            out=dbu4[:, :, 0:SEQ],
                            in0=wdu[m].unsqueeze(1).broadcast_to([P, NG, SEQ]),
                            in1=bcg[:, 0, :, :], op=OP.mult)
                        for j in range(NG):
                            nc.scalar.activation(out=da4[:, j, 0:SEQ],
                                                 in_=delta_all[:, m, :],
                                                 func=AF.Exp,
                                                 scale=A_sb[:, m, n0 + j:n0 + j + 1])
                        h4 = sc_pool.tile([P, NG, SP2], BF16, tag="h")
                        nc.vector.tensor_tensor_scan(
                            out=h4.rearrange("p a b -> p (a b)"),
                            data0=da4.rearrange("p a b -> p (a b)"),
                            data1=dbu4.rearrange("p a b -> p (a b)"),
                            initial=0.0, op0=OP.mult, op1=OP.add)
                        g4 = sc_pool.tile([P, NG, SEQ], BF16, tag="g")
                        nc.vector.tensor_tensor(out=g4, in0=h4[:, :, 0:SEQ],
                                                in1=bcg[:, 1, :, :], op=OP.mult)
                        for j in range(NG):
                            for th in range(TH):
                                nc.tensor.matmul(
                                    yps[i][:, th * 512:(th + 1) * 512], eye_b,
                                    g4[:, j, th * 512:(th + 1) * 512],
                                    start=(n0 + j == 0), stop=False)
                # Phase C for this d-set: y += u*D on PE, then gate with silu(z)
                for i in range(DPS):
                    m = ds * DPS + i
                    dD = sc_pool.tile([P, P], F32R, tag="dD", bufs=3)
                    nc.vector.tensor_scalar_mul(dD, eye_sb, dsk_sb[:, m:m + 1])
                    for th in range(TH):
                        nc.tensor.matmul(yps[i][:, th * 512:(th + 1) * 512], dD,
                                         us[m][:, th * 512:(th + 1) * 512],
                                         start=False, stop=True)
                    nc.vector.tensor_tensor(out=yf[m], in0=yps[i], in1=sz[m],
                                            op=OP.mult)

        # ---------------- Phase D: out_proj ----------------
        with ExitStack() as pd:
            ost = pd.enter_context(tc.tile_pool(name="ost", bufs=2))
            for m in range(KM):
                ot = ost.tile([P, SEQ], F32, tag="ot")
                for th in range(TH):
                    ps = ops_pool.tile([P, 512], F32, tag="ps")
                    for k in range(MH):
                        nc.tensor.matmul(ps, ow_sb[:, k, m, :],
                                         yf[k][:, th * 512:(th + 1) * 512],
                                         start=(k == 0), stop=(k == MH - 1))
                    nc.scalar.copy(out=ot[:, th * 512:(th + 1) * 512], in_=ps)
                nc.sync.dma_start(out=outp[m * P:(m + 1) * P, :], in_=ot)

    nc.finalize()
    return nc


def _prep_core(x, prm, b, direction, half):
    """Build the per-core input map. prm maps param name -> array."""
    xb = np.ascontiguousarray(x[b])                # (L, D_MODEL)
    if direction == 1:
        xb = np.ascontiguousarray(xb[::-1])
    in_w = prm["in_w"]
    conv_w = prm["conv_w"]
    conv_b = prm["conv_b"]
    xproj_w = prm["xproj_w"]
    dt_w = prm["dt_w"]
    dt_b = prm["dt_b"]
    Alog = prm["Alog"]
    Dp = prm["D"]
    out_w = prm["out_w"]

    own = np.arange(half * DH, (half + 1) * DH)
    oth = np.arange((1 - half) * DH, (2 - half) * DH)
    perm = np.concatenate([own, oth])              # u-channel permutation

    wu = in_w[0:D_INNER][perm]                     # (1536, 768), own half first
    wz = in_w[D_INNER:2 * D_INNER][own]            # (768, 768)
    cw = conv_w[perm]                              # (1536, 4)
    A = -np.exp(Alog[own])                         # (768, 16)

    def lhs_tiles(mat_t, kk, mm):
        # (K*P, M*P) -> (mm, P, kk*P): per m-tile, partition-contiguous rows
        return np.ascontiguousarray(
            mat_t.reshape(kk, P, mm, P).transpose(2, 1, 0, 3).reshape(mm, P, kk * P))

    return {
        "xT": np.ascontiguousarray(xb.T.reshape(KM, P, SEQ).transpose(1, 0, 2)),
        "wuX": lhs_tiles(wu.T, KM, MU),
        "wzX": lhs_tiles(wz.T, KM, MH),
        "convw": np.ascontiguousarray(cw.reshape(MU, P, D_CONV).transpose(1, 0, 2)),
        "cbias": np.ascontiguousarray(conv_b[perm].reshape(MU, P).T),
        "xpX": np.ascontiguousarray(
            xproj_w[:, perm].T.reshape(MU, P, 80).transpose(1, 0, 2)),
        "dtwT": np.ascontiguousarray(
            np.vstack([dt_w[own].T, dt_b[own][None, :]])),
        "ones1": np.ones((1, SEQ), dtype=np.float32),
        "Amat": np.ascontiguousarray(A.reshape(MH, P, D_STATE).transpose(1, 0, 2)),
        "Dsk": np.ascontiguousarray(Dp[own].reshape(MH, P).T),
        "owX": np.ascontiguousarray(
            out_w[:, own].T.reshape(MH, P, KM, P).transpose(1, 0, 2, 3)),
        "eye": np.eye(P, dtype=np.float32),
        "zpad": np.zeros((P, D_CONV - 1), dtype=np.float32),
        "zb": np.zeros((P, 2), dtype=ml_dtypes.bfloat16),
    }


def _in_maps(inputs):
    x = inputs["x"]
    maps = []
    for b in range(BATCH):
        for direction in range(2):
            pfx = "f" if direction == 0 else "b"
            prm = {k: inputs[f"{pfx}_{k}"] for k in
                   ("in_w", "conv_w", "conv_b", "xproj_w", "dt_w", "dt_b",
                    "Alog", "D", "out_w")}
            for half in range(2):
                maps.append(_prep_core(x, prm, b, direction, half))
    return maps


def kernel(**inputs):
    inputs = {k: np.asarray(v, dtype=np.float32) for k, v in inputs.items()}
    nc = _CACHE.get("nc")
    if nc is None:
        nc = _build()
        _CACHE["nc"] = nc
    maps = _in_maps(inputs)
    res = run_bass_kernel_spmd(nc, maps, list(range(8)),
                               **_CACHE.get("run_kwargs", {}))
    _CACHE["last_results"] = res
    out = np.zeros((BATCH, SEQ, D_MODEL), dtype=np.float32)
    ci = 0
    for b in range(BATCH):
        for direction in range(2):
            for half in range(2):
                part = res.results[ci]["outp"].T          # (SEQ, D_MODEL)
                if direction == 1:
                    part = part[::-1]
                out[b] += part
                ci += 1
    return out


# revision 25
# speedup vs baseline: 1.0269x; 1.0206x over previous
"""Bidirectional Mamba layer for Trainium2 (8 NeuronCores).

Sharding: core = (batch b in {0,1}) x (direction in {fwd,bwd}) x (d_inner half).
All 8 cores run one SPMD program with per-core input arrays; there are no
cross-core collectives. The host flips the sequence for the backward direction,
permutes u-channels so each core's own d_inner half is always channel-tiles
0..5, pre-tiles every weight matrix so each SBUF destination loads with one
large contiguous DMA (the HWDGE unit costs ~625ns per DMA instruction), and
sums the row-parallel + fwd/bwd partial outputs during the gather.

Per-core program:
  A) in_proj (fp32r matmuls), causal depthwise conv as 4 diagonal-matmul taps
     on the tensor engine (diagonals built on the idle vector engine), SiLU;
     xproj accumulated incrementally as each u-tile is produced;
     softplus(dt_proj + bias) via exp+ln; w = delta*u.
  B) selective scan: for each (d-tile, state n): dA = exp(delta * A[:,n]) on
     the scalar engine, dBu = w * bcast(B_n) on vector, hardware
     tensor_tensor_scan over t, g = h * bcast(C_n), and y += I.T @ g
     accumulated in PSUM by the tensor engine (the sum over n).
  C) y = (y + u*D) * silu(z);  D) out_proj partial, summed on host.
"""
import sys

sys.path.insert(0, "/opt/trn_rl_repo")

from contextlib import ExitStack

import ml_dtypes
import numpy as np

import concourse.bass as bass
import concourse.mybir as mybir
import concourse.tile as tile
from concourse import bacc
from concourse.bass_utils import run_bass_kernel_spmd

D_MODEL = 768
D_STATE = 16
D_INNER = 1536
DT_RANK = 48
D_CONV = 4
BATCH = 2
SEQ = 1024
DH = D_INNER // 2          # 768 scan channels per core
P = 128
KM = D_MODEL // P          # 6 k-tiles over d_model
MU = D_INNER // P          # 12 m-tiles for full u
MH = DH // P               # 6 m-tiles for the half (z, delta, scan, out_proj k)
TH = SEQ // 512            # 2 t-halves for matmul free dim

F32 = mybir.dt.float32
F32R = mybir.dt.float32r
BF16 = mybir.dt.bfloat16
AF = mybir.ActivationFunctionType
OP = mybir.AluOpType

_CACHE = {}


def _build():
    nc = bacc.Bacc("TRN2", target_bir_lowering=False, debug=False)

    xT = nc.dram_tensor("xT", [P, KM, SEQ], F32R, kind="ExternalInput")
    wuX = nc.dram_tensor("wuX", [MU, P, KM * P], F32R, kind="ExternalInput")
    wzX = nc.dram_tensor("wzX", [MH, P, KM * P], F32R, kind="ExternalInput")
    convw = nc.dram_tensor("convw", [P, MU, D_CONV], F32, kind="ExternalInput")
    cbias = nc.dram_tensor("cbias", [P, MU], F32, kind="ExternalInput")
    xpX = nc.dram_tensor("xpX", [P, MU, 80], F32R, kind="ExternalInput")
    dtwT = nc.dram_tensor("dtwT", [DT_RANK + 1, DH], F32R, kind="ExternalInput")
    ones1 = nc.dram_tensor("ones1", [1, SEQ], F32R, kind="ExternalInput")
    Amat = nc.dram_tensor("Amat", [P, MH, D_STATE], F32, kind="ExternalInput")
    Dsk = nc.dram_tensor("Dsk", [P, MH], F32, kind="ExternalInput")
    owX = nc.dram_tensor("owX", [P, MH, KM, P], F32R, kind="ExternalInput")
    eye = nc.dram_tensor("eye", [P, P], F32R, kind="ExternalInput")
    zpad = nc.dram_tensor("zpad", [P, D_CONV - 1], F32R, kind="ExternalInput")
    zb = nc.dram_tensor("zb", [P, 2], BF16, kind="ExternalInput")
    outp = nc.dram_tensor("outp", [D_MODEL, SEQ], F32, kind="ExternalOutput")

    with tile.TileContext(nc) as tc, ExitStack() as top:
        persist = top.enter_context(tc.tile_pool(name="persist", bufs=1))
        ops_pool = top.enter_context(tc.tile_pool(name="ps_o", bufs=2, space="PSUM"))
        dram = top.enter_context(tc.tile_pool(name="dram", bufs=1, space="DRAM"))
        us = [persist.tile([P, SEQ], F32R, tag=f"us{m}", name=f"us{m}")
              for m in range(MH)]
        sz = [persist.tile([P, SEQ], F32, tag=f"sz{m}", name=f"sz{m}")
              for m in range(MH)]
        delta_all = persist.tile([P, MH, SEQ], BF16, tag="dl")
        wdu = [persist.tile([P, SEQ], BF16, tag=f"w{m}", name=f"w{m}")
               for m in range(MH)]
        A_sb = persist.tile([P, MH, D_STATE], F32, tag="A")
        cb_sb = persist.tile([P, MU], F32, tag="cb")
        dsk_sb = persist.tile([P, MH], F32, tag="dsk")
        cw_sb = persist.tile([P, MU, D_CONV], F32, tag="cw")
        eye_sb = persist.tile([P, P], F32R, tag="eye")
        ow_sb = persist.tile([P, MH, KM, P], F32R, tag="ow")
        eye_b = persist.tile([P, P], BF16, tag="eyeb")
        bcd = dram.tile([2 * D_STATE, SEQ], BF16, tag="bc")
        nc.sync.dma_start(out=A_sb, in_=Amat[:, :, :])
        nc.sync.dma_start(out=dsk_sb, in_=Dsk[:, :])
        nc.sync.dma_start(out=cb_sb, in_=cbias[:, :])
        nc.sync.dma_start(out=cw_sb, in_=convw[:, :, :])
        nc.sync.dma_start(out=eye_sb, in_=eye[:, :])

        # ---------------- Phase A: projections ----------------
        with ExitStack() as pa:
            xs_pool = top.enter_context(tc.tile_pool(name="xs", bufs=1))
            wpool = top.enter_context(tc.tile_pool(name="wstream", bufs=4))
            djpool = pa.enter_context(tc.tile_pool(name="djp", bufs=8))
            ubuf_pool = pa.enter_context(tc.tile_pool(name="ubuf", bufs=1))
            uoth_pool = pa.enter_context(tc.tile_pool(name="uoth", bufs=2))
            ps_a = pa.enter_context(tc.tile_pool(name="ps_a", bufs=2, space="PSUM"))
            ps_xp = pa.enter_context(tc.tile_pool(name="ps_xp", bufs=1, space="PSUM"))
            misc = pa.enter_context(tc.tile_pool(name="misc_a", bufs=1))

            xs_all = xs_pool.tile([P, KM, SEQ], F32R, tag="xs")
            xs = [xs_all[:, k, :] for k in range(KM)]
            # first x chunk and first weight tile land before the rest so the
            # tensor engine starts early
            nc.sync.dma_start(out=xs_all[:, 0, :], in_=xT[:, 0, :])
            wu0 = wpool.tile([P, KM * P], F32R, tag="w")
            nc.sync.dma_start(out=wu0, in_=wuX[0, :, :])
            for k in range(1, KM):
                nc.sync.dma_start(out=xs_all[:, k, :], in_=xT[:, k, :])

            xp_all = misc.tile([P, MU, 80], F32R, tag="xp")
            nc.sync.dma_start(out=xp_all, in_=xpX[:, :, :])

            # two conv staging buffers; zero pad written once each
            ubufs = [ubuf_pool.tile([P, D_CONV - 1 + SEQ], F32R, tag=f"ubuf{i}",
                                    name=f"ubuf{i}") for i in range(2)]
            for i in range(2):
                nc.sync.dma_start(out=ubufs[i][:, 0:D_CONV - 1], in_=zpad[:, :])

            # xproj accumulators, fed incrementally as each u-tile is made
            psx = [ps_xp.tile([80, 512], F32, tag=f"psx{th}", name=f"psx{th}")
                   for th in range(TH)]

            # u path: in_proj -> causal conv -> silu -> xproj contribution
            for m in range(MU):
                if m == 0:
                    wu_m = wu0
                else:
                    wu_m = wpool.tile([P, KM * P], F32R, tag="w")
                    nc.sync.dma_start(out=wu_m, in_=wuX[m, :, :])
                ub = ubufs[m % 2]
                for th in range(TH):
                    ps = ps_a.tile([P, 512], F32, tag="ps")
                    for k in range(KM):
                        nc.tensor.matmul(ps, wu_m[:, k * P:(k + 1) * P],
                                         xs[k][:, th * 512:(th + 1) * 512],
                                         start=(k == 0), stop=(k == KM - 1))
                    nc.scalar.copy(
                        out=ub[:, D_CONV - 1 + th * 512:D_CONV - 1 + (th + 1) * 512],
                        in_=ps)
                # depthwise causal conv as 4 diagonal-matmul taps;
                # diagonals built on the (idle) vector engine
                ut = us[m] if m < MH else uoth_pool.tile([P, SEQ], F32R,
                                                         tag="uo", name="uo")
                djs = []
                for j in range(D_CONV):
                    dj = djpool.tile([P, P], F32R, tag="dj")
                    nc.vector.tensor_scalar_mul(dj, eye_sb, cw_sb[:, m, j:j + 1])
                    djs.append(dj)
                for th in range(TH):
                    psc = ps_a.tile([P, 512], F32, tag="ps")
                    for j in range(D_CONV):
                        nc.tensor.matmul(psc, djs[j],
                                         ub[:, j + th * 512:j + th * 512 + 512],
                                         start=(j == 0), stop=(j == D_CONV - 1))
                    nc.scalar.activation(out=ut[:, th * 512:(th + 1) * 512], in_=psc,
                                         func=AF.Silu, bias=cb_sb[:, m:m + 1])
                # xproj: accumulate this k=m contribution into psx
                for th in range(TH):
                    nc.tensor.matmul(psx[th], xp_all[:, m, :],
                                     ut[:, th * 512:(th + 1) * 512],
                                     start=(m == 0), stop=(m == MU - 1))

            # z path: in_proj half + silu (PE fills the delta/ACT window)
            for mz in range(MH):
                wz_m = wpool.tile([P, KM * P], F32R, tag="w")
                nc.sync.dma_start(out=wz_m, in_=wzX[mz, :, :])
                for th in range(TH):
                    ps = ops_pool.tile([P, 512], F32, tag="ps")
                    for k in range(KM):
                        nc.tensor.matmul(ps, wz_m[:, k * P:(k + 1) * P],
                                         xs[k][:, th * 512:(th + 1) * 512],
                                         start=(k == 0), stop=(k == KM - 1))
                    nc.scalar.activation(out=sz[mz][:, th * 512:(th + 1) * 512],
                                         in_=ps, func=AF.Silu)

            # x_dbl out of PSUM: fp32 copy (B/C rows) + fp32r copy (dt rows)
            xd_bc = misc.tile([80, SEQ], BF16, tag="xdbc")
            xd_r = misc.tile([DT_RANK + 1, SEQ], F32R, tag="xdr")
            for th in range(TH):
                # non-zero-base partition slices are limited to 32 partitions
                nc.scalar.copy(out=xd_bc[32:64, th * 512:(th + 1) * 512],
                               in_=psx[th][32:64, :])
                nc.scalar.copy(out=xd_bc[64:80, th * 512:(th + 1) * 512],
                               in_=psx[th][64:80, :])
                nc.scalar.copy(out=xd_r[0:DT_RANK, th * 512:(th + 1) * 512],
                               in_=psx[th][0:DT_RANK, :])

            # delta = softplus(dt @ dt_w.T + dt_b) = ln(exp(.) + 1), batched:
            # dt_b rides as an extra contraction row against a ones-row, so
            # exp/ln run as two whole-width ACT ops (no table thrash)
            nc.sync.dma_start(out=xd_r[DT_RANK:DT_RANK + 1, :], in_=ones1[:, :])
            dtw_sb = misc.tile([DT_RANK + 1, DH], F32R, tag="dtw")
            nc.sync.dma_start(out=dtw_sb, in_=dtwT[:, :])
            ps_dt = pa.enter_context(tc.tile_pool(name="ps_dt", bufs=1,
                                                  space="PSUM"))
            for th in range(TH):
                e1 = misc.tile([P, MH, 512], BF16, tag="sp_e", bufs=2)
                for mb in range(MH // 2):
                    psd2 = ps_dt.tile([P, 2, 512], F32, tag="psd")
                    for mi in range(2):
                        m = 2 * mb + mi
                        nc.tensor.matmul(psd2[:, mi, :],
                                         dtw_sb[:, m * P:(m + 1) * P],
                                         xd_r[:, th * 512:(th + 1) * 512],
                                         start=True, stop=True)
                    nc.scalar.activation(out=e1[:, 2 * mb:2 * mb + 2, :],
                                         in_=psd2, func=AF.Exp)
                nc.scalar.activation(
                    out=delta_all[:, :, th * 512:(th + 1) * 512],
                    in_=e1, func=AF.Ln, bias=1.0)

            # w = delta * u  (scan-half channels only)
            for m in range(MH):
                nc.vector.tensor_tensor(out=wdu[m], in0=delta_all[:, m, :],
                                        in1=us[m], op=OP.mult)

            # stage B and C rows to DRAM for partition-broadcast reads
            nc.sync.dma_start(out=bcd[:, :], in_=xd_bc[DT_RANK:80, :])

        nc.sync.dma_start(out=ow_sb, in_=owX[:, :, :, :])
        nc.scalar.copy(out=eye_b, in_=eye_sb)

        late = top.enter_context(tc.tile_pool(name="late", bufs=1))
        yf = [late.tile([P, SEQ], F32R, tag=f"yf{m}", name=f"yf{m}")
              for m in range(MH)]

        # ---------------- Phase B: selective scan ----------------
        _CACHE0 = {}
        with ExitStack() as pb:
            bc_pool = pb.enter_context(tc.tile_pool(name="bc", bufs=2))
            sc_pool = pb.enter_context(tc.tile_pool(name="scan", bufs=2))
            ps_y = pb.enter_context(tc.tile_pool(name="ps_y", bufs=1, space="PSUM"))
            NDSET = 2
            DPS = MH // NDSET  # 3 d-tiles per set
            for ds in range(NDSET):
                yps = [ps_y.tile([P, SEQ], F32, tag=f"y{i}", name=f"y{i}")
                       for i in range(DPS)]
                NG = 2
                for np_ in range(D_STATE // NG):
                    n0 = NG * np_
                    # rows {n0..n0+3} and {16+n0..}: [bc-pair, n-group, t]
                    bcg = bc_pool.tile([P, 2, NG, SEQ], BF16, tag="bc2")
                    srcg = bass.AP(
                        tensor=bcd.tensor, offset=bcd.offset + n0 * SEQ,
                        ap=[[0, P], [D_STATE * SEQ, 2], [SEQ, NG], [1, SEQ]])
                    nc.sync.dma_start(out=bcg, in_=srcg)
                    for i in range(DPS):
                        m = ds * DPS + i
                        # rows padded to SEQ+2 with zero boundary columns so a
                        # single chained scan covers both n's (state resets to
                        # zero through the dA=0, dBu=0 boundary elements);
                        # even row stride keeps bf16 ops 4B-aligned
                        SP2 = SEQ + 2
                        dbu4 = sc_pool.tile([P, NG, SP2], BF16, tag="dbu")
                        da4 = sc_pool.tile([P, NG, SP2], BF16, tag="da")
                        ctr = _CACHE0.setdefault("bz", 0)
                        if ctr < 2:
                            _CACHE0["bz"] = ctr + 1
                            for tzi in (dbu4, da4):
                                nc.sync.dma_start(
                                    out=tzi[:, :, SEQ:SP2],
                                    in_=zb[:, :].unsqueeze(1)
                                        .broadcast_to([P, NG, 2]))
                        nc.vector.tensor_tensor(
                            out=dbu4[:, :, 0:SEQ],
                            in0=wdu[m].unsqueeze(1).broadcast_to([P, NG, SEQ]),
                            in1=bcg[:, 0, :, :], op=OP.mult)
                        for j in range(NG):
                            nc.scalar.activation(out=da4[:, j, 0:SEQ],
                                                 in_=delta_all[:, m, :],
                                                 func=AF.Exp,
                                                 scale=A_sb[:, m, n0 + j:n0 + j + 1])
                        h4 = sc_pool.tile([P, NG, SP2], BF16, tag="h")
                        nc.vector.tensor_tensor_scan(
                            out=h4.rearrange("p a b -> p (a b)"),
                            data0=da4.rearrange("p a b -> p (a b)"),
                            data1=dbu4.rearrange("p a b -> p (a b)"),
                            initial=0.0, op0=OP.mult, op1=OP.add)
                        g4 = sc_pool.tile([P, NG, SEQ], BF16, tag="g")
                        nc.vector.tensor_tensor(out=g4, in0=h4[:, :, 0:SEQ],
                                                in1=bcg[:, 1, :, :], op=OP.mult)
                        for j in range(NG):
                            for th in range(TH):
                                nc.tensor.matmul(
                                    yps[i][:, th * 512:(th + 1) * 512], eye_b,
                                    g4[:, j, th * 512:(th + 1) * 512],
                                    start=(n0 + j == 0), stop=False)
                # Phase C for this d-set: y += u*D on PE, then gate with silu(z)
                for i in range(DPS):
                    m = ds * DPS + i
                    dD = sc_pool.tile([P, P], F32R, tag="dD", bufs=3)
                    nc.vector.tensor_scalar_mul(dD, eye_sb, dsk_sb[:, m:m + 1])
                    for th in range(TH):
                        nc.tensor.matmul(yps[i][:, th * 512:(th + 1) * 512], dD,
                                         us[m][:, th * 512:(th + 1) * 512],
                                         start=False, stop=True)
                    nc.vector.tensor_tensor(out=yf[m], in0=yps[i], in1=sz[m],
                                            op=OP.mult)

        # ---------------- Phase D: out_proj ----------------
        with ExitStack() as pd:
            ost = pd.enter_context(tc.tile_pool(name="ost", bufs=2))
            for m in range(KM):
                ot = ost.tile([P, SEQ], F32, tag="ot")
                for th in range(TH):
                    ps = ops_pool.tile([P, 512], F32, tag="ps")
                    for k in range(MH):
                        nc.tensor.matmul(ps, ow_sb[:, k, m, :],
                                         yf[k][:, th * 512:(th + 1) * 512],
                                         start=(k == 0), stop=(k == MH - 1))
                    nc.scalar.copy(out=ot[:, th * 512:(th + 1) * 512], in_=ps)
                nc.sync.dma_start(out=outp[m * P:(m + 1) * P, :], in_=ot)

    nc.finalize()
    return nc


def _prep_core(x, prm, b, direction, half):
    """Build the per-core input map. prm maps param name -> array."""
    xb = np.ascontiguousarray(x[b])                # (L, D_MODEL)
    if direction == 1:
        xb = np.ascontiguousarray(xb[::-1])
    in_w = prm["in_w"]
    conv_w = prm["conv_w"]
    conv_b = prm["conv_b"]
    xproj_w = prm["xproj_w"]
    dt_w = prm["dt_w"]
    dt_b = prm["dt_b"]
    Alog = prm["Alog"]
    Dp = prm["D"]
    out_w = prm["out_w"]

    own = np.arange(half * DH, (half + 1) * DH)
    oth = np.arange((1 - half) * DH, (2 - half) * DH)
    perm = np.concatenate([own, oth])              # u-channel permutation

    wu = in_w[0:D_INNER][perm]                     # (1536, 768), own half first
    wz = in_w[D_INNER:2 * D_INNER][own]            # (768, 768)
    cw = conv_w[perm]                              # (1536, 4)
    A = -np.exp(Alog[own])                         # (768, 16)

    def lhs_tiles(mat_t, kk, mm):
        # (K*P, M*P) -> (mm, P, kk*P): per m-tile, partition-contiguous rows
        return np.ascontiguousarray(
            mat_t.reshape(kk, P, mm, P).transpose(2, 1, 0, 3).reshape(mm, P, kk * P))

    return {
        "xT": np.ascontiguousarray(xb.T.reshape(KM, P, SEQ).transpose(1, 0, 2)),
        "wuX": lhs_tiles(wu.T, KM, MU),
        "wzX": lhs_tiles(wz.T, KM, MH),
        "convw": np.ascontiguousarray(cw.reshape(MU, P, D_CONV).transpose(1, 0, 2)),
        "cbias": np.ascontiguousarray(conv_b[perm].reshape(MU, P).T),
        "xpX": np.ascontiguousarray(
            xproj_w[:, perm].T.reshape(MU, P, 80).transpose(1, 0, 2)),
        "dtwT": np.ascontiguousarray(
            np.vstack([dt_w[own].T, dt_b[own][None, :]])),
        "ones1": np.ones((1, SEQ), dtype=np.float32),
        "Amat": np.ascontiguousarray(A.reshape(MH, P, D_STATE).transpose(1, 0, 2)),
        "Dsk": np.ascontiguousarray(Dp[own].reshape(MH, P).T),
        "owX": np.ascontiguousarray(
            out_w[:, own].T.reshape(MH, P, KM, P).transpose(1, 0, 2, 3)),
        "eye": np.eye(P, dtype=np.float32),
        "zpad": np.zeros((P, D_CONV - 1), dtype=np.float32),
        "zb": np.zeros((P, 2), dtype=ml_dtypes.bfloat16),
    }


def _in_maps(inputs):
    x = inputs["x"]
    maps = []
    for b in range(BATCH):
        for direction in range(2):
            pfx = "f" if direction == 0 else "b"
            prm = {k: inputs[f"{pfx}_{k}"] for k in
                   ("in_w", "conv_w", "conv_b", "xproj_w", "dt_w", "dt_b",
                    "Alog", "D", "out_w")}
            for half in range(2):
                maps.append(_prep_core(x, prm, b, direction, half))
    return maps


def kernel(**inputs):
    inputs = {k: np.asarray(v, dtype=np.float32) for k, v in inputs.items()}
    nc = _CACHE.get("nc")
    if nc is None:
        nc = _build()
        _CACHE["nc"] = nc
    maps = _in_maps(inputs)
    res = run_bass_kernel_spmd(nc, maps, list(range(8)),
                               **_CACHE.get("run_kwargs", {}))
    _CACHE["last_results"] = res
    out = np.zeros((BATCH, SEQ, D_MODEL), dtype=np.float32)
    ci = 0
    for b in range(BATCH):
        for direction in range(2):
            for half in range(2):
                part = res.results[ci]["outp"].T          # (SEQ, D_MODEL)
                if direction == 1:
                    part = part[::-1]
                out[b] += part
                ci += 1
    return out
